# revision 48
# baseline (speedup 1.0000x reference)
"""4-layer GraphSAGE (mean aggr) on 8 TRN2 NeuronCores — gather + PE segsum.

Strategy (dst-owner node partitioning, no scatter):
  - Nodes partitioned across 8 cores (12500 each, padded to 12544 = 98*128).
    Each core owns 98 dst blocks of 128 nodes.
  - Per layer, a replicated DRAM table holds TRANSFORMED features
    (table_l = h_{l-1} @ Wl_l, bf16), built shard-wise and AllGathered.
    Gathering transformed 128-wide rows makes the segment-sum directly
    produce mean@Wl.
  - Edges are grouped host-side by (dst block, src range) cells — 4 ranges
    of 25088 table rows keep gather indices int16. Each cell is padded to
    CAPG groups of 128 edges.
  - Aggregation is a tensor-engine segment-sum: for each 128-edge group, a
    one-hot matrix S[e, d] = (dst_rel[e] == d) * invdeg[e] is built on the
    vector engine (one is_equal against a replicated iota + one multiply),
    then matmul-accumulated into PSUM. No dma_scatter_add, no WAW chains.
  - Layers 1-3 accumulate feature-major psum [f, dst]: segsum (G as lhsT)
    + self term (Wr as lhsT, hT_prev as rhs), evicted with fused
    bias+ReLU (Act engine, per-partition bias) straight into an SBUF-resident
    feature-major hT. The next table (h @ Wl_{l+1}) is one more matmul per
    block (hT as lhsT) producing node-major rows — zero PE transposes.
  - Layer 4 accumulates node-major [dst, f] (S as lhsT); bias is a rank-1
    matmul (ones x b4^T); evicted fp32 to the output.
"""

import numpy as np

# ---------------------------------------------------------------- constants
NCORES = 8
N = 100000
E = 1600000
F_IN = 16
H = 128
SHARD = 12500            # real nodes owned per core
BLK = 128
NBLK = 98                # 98*128 = 12544
SHARD_P = NBLK * BLK     # padded shard rows
TBL_ROWS = NCORES * SHARD_P   # 100352
NRANGE = 4
RANGE_ROWS = TBL_ROWS // NRANGE  # 25088 (< 2**15)
# Table rows are stored half-major so each AllGather covers a contiguous
# half: srow = half*50176 + core*6272 + (local - half*6272), half = blocks
# 0-48 vs 49-97 of the owning core's shard. Ranges 0/1 = half A, 2/3 = half B.
HBLK = 49                     # dst blocks per table half
HALF_ROWS = HBLK * BLK        # 6272 rows contributed per core per half
HALF_TBL = NCORES * HALF_ROWS  # 50176 = 2 * RANGE_ROWS
# Gather groups (of 128 edges) per gather instruction. num_idxs > 1024 hangs
# the device (hard HW limit), so 8 groups = 1024 idxs per instruction (last
# chunk per range is partial). The Q7 descriptor generation (~7ns/desc,
# serial per queue) dominates gather cost, so the 4 ranges are spread over
# 4 SWDGE queues (separate Q7 core pairs).
NG_CHUNK = 8

_compiled = {}


# ---------------------------------------------------------------- program
def _build_program(capg, repeat=1, no_dve=False, no_cc=False, no_gather=False):
    import concourse.bacc as bacc
    import concourse.mybir as mybir
    import concourse.tile as tile

    fp32 = mybir.dt.float32
    bf16 = mybir.dt.bfloat16
    i16 = mybir.dt.int16
    AF = mybir.ActivationFunctionType
    EQ = mybir.AluOpType.is_equal

    GPB = NRANGE * capg          # groups per block (across ranges)
    NGRP = NBLK * GPB            # total groups (= dr/iv columns)
    SGRP_R = NBLK * capg         # groups per range stream
    NCHUNKS_R = -(-SGRP_R // NG_CHUNK)   # last chunk may be partial
    IDXC_R = SGRP_R * 8          # idx cols (16-wide wrap) per range

    def chunk_groups(c):
        return min(NG_CHUNK, SGRP_R - c * NG_CHUNK)

    nc = bacc.Bacc(
        "TRN2",
        target_bir_lowering=False,
        debug=False,
        enable_asserts=False,
        num_devices=NCORES,
        num_swdge_queues=4,
    )

    # -------- I/O declarations
    xt_d = nc.dram_tensor("xt", [F_IN, SHARD_P], bf16, kind="ExternalInput")
    idx_d = nc.dram_tensor("idx", [128, NRANGE * IDXC_R], i16, kind="ExternalInput")
    dr_d = nc.dram_tensor("dr", [128, NGRP], bf16, kind="ExternalInput")
    iv_d = nc.dram_tensor("iv", [128, NGRP], bf16, kind="ExternalInput")
    iota_d = nc.dram_tensor("iota", [128, 128], bf16, kind="ExternalInput")
    # layer-1 table (x @ Wl1, half-major row layout) is a pure function of
    # the inputs — precomputed host-side and shipped replicated, so layer-1
    # gathers have no on-device producer and start immediately.
    t1_d = [
        nc.dram_tensor("t1a", [HALF_TBL, H], bf16, kind="ExternalInput"),
        nc.dram_tensor("t1b", [HALF_TBL, H], bf16, kind="ExternalInput"),
    ]
    w_d = {}
    for l in range(1, 5):
        din = F_IN if l == 1 else H
        w_d[f"wl{l}"] = nc.dram_tensor(f"wl{l}", [din, H], bf16, kind="ExternalInput")
        w_d[f"wr{l}"] = nc.dram_tensor(f"wr{l}", [din, H], bf16, kind="ExternalInput")
    for l in range(1, 4):
        w_d[f"bc{l}"] = nc.dram_tensor(f"bc{l}", [128, 1], fp32, kind="ExternalInput")
    w_d["b4b"] = nc.dram_tensor("b4b", [128, 128], fp32, kind="ExternalInput")

    out_d = nc.dram_tensor("out", [SHARD_P, H], fp32, kind="ExternalOutput")

    with tile.TileContext(nc) as tc:
        with (
            tc.tile_pool(name="dram", bufs=1, space="DRAM") as dpool,
            tc.tile_pool(name="const", bufs=1) as cpool,
            tc.tile_pool(name="g0", bufs=8) as gp0,
            tc.tile_pool(name="g1", bufs=8) as gp1,
            tc.tile_pool(name="g2", bufs=6) as gp2,
            tc.tile_pool(name="g3", bufs=6) as gp3,
            tc.tile_pool(name="sp", bufs=4) as spool,
            tc.tile_pool(name="work", bufs=4) as wpool,
            tc.tile_pool(name="psum_a", bufs=6, space="PSUM") as popool,
            tc.tile_pool(name="psum_t", bufs=2, space="PSUM") as ptpool,
        ):
            gpools = [gp0, gp1, gp2, gp3]
            # Shared tiles may only be written by one instruction each, so
            # tables (AllGather outputs) are allocated per repeat. Each
            # layer's table is two Shared halves (separate AllGathers).
            tbls_r = [
                [[dpool.tile([HALF_TBL, H], bf16, addr_space="Shared",
                             name=f"tbl{l}{hn}_r{rep}")
                  for hn in ("a", "b")]
                 for l in range(4)]
                for rep in range(repeat)
            ]
            shs_r = [
                [[dpool.tile([HALF_ROWS, H], bf16, name=f"sh{l}{hn}_r{rep}")
                  for hn in ("a", "b")]
                 for l in range(4)]
                for rep in range(repeat)
            ]

            # -------- constants to SBUF
            idx_sb = cpool.tile([128, NRANGE * IDXC_R], i16, name="idx_sb")
            nc.sync.dma_start(idx_sb[:], idx_d.ap())
            dr_sb = cpool.tile([128, NGRP], bf16, name="dr_sb")
            nc.sync.dma_start(dr_sb[:], dr_d.ap())
            iv_sb = cpool.tile([128, NGRP], bf16, name="iv_sb")
            nc.sync.dma_start(iv_sb[:], iv_d.ap())
            iota1 = cpool.tile([128, 128], bf16, name="iota1")
            nc.sync.dma_start(iota1[:], iota_d.ap())
            iota20 = cpool.tile([128, GPB, 128], bf16, name="iota20")
            for j in range(GPB):
                nc.vector.tensor_copy(iota20[:, j, :], iota1[:])
            xt_sb = cpool.tile([F_IN, SHARD_P], bf16, name="xt_sb")
            nc.sync.dma_start(xt_sb[:], xt_d.ap())
            w_sb = {}
            for l in range(1, 5):
                din = F_IN if l == 1 else H
                for nm in (f"wl{l}", f"wr{l}"):
                    t = cpool.tile([din, H], bf16, name=f"{nm}_sb")
                    nc.sync.dma_start(t[:], w_d[nm].ap())
                    w_sb[nm] = t
            for l in range(1, 4):
                t = cpool.tile([128, 1], fp32, name=f"bc{l}_sb")
                nc.sync.dma_start(t[:], w_d[f"bc{l}"].ap())
                w_sb[f"bc{l}"] = t
            b4b_sb = cpool.tile([128, 128], fp32, name="b4b_sb")
            nc.sync.dma_start(b4b_sb[:], w_d["b4b"].ap())

            # feature-major hidden state, SBUF resident, ping-pong
            hA = cpool.tile([128, SHARD_P], bf16, name="hA")
            hB = cpool.tile([128, SHARD_P], bf16, name="hB")

            def nm_view(t):
                return t.rearrange("(b p) f -> p b f", p=128)

            out_v = nm_view(out_d.ap())

            def allgather(src, dst):
                if no_cc:
                    nc.sync.dma_start(dst[:HALF_ROWS, :], src[:, :])
                    return
                nc.gpsimd.collective_compute(
                    "AllGather",
                    mybir.AluOpType.bypass,
                    replica_groups=[list(range(NCORES))],
                    ins=[src.opt()],
                    outs=[dst.opt()],
                )

            def range_slice(tbl_halves, r):
                # ranges 0/1 live in half A, 2/3 in half B
                half = tbl_halves[r // 2]
                lo = (r % 2) * RANGE_ROWS
                return half[lo : lo + RANGE_ROWS, :]

            def emit_gather(tbl_halves, gt, r, c):
                gic = chunk_groups(c)
                G = gpools[r].tile([128, NG_CHUNK, 128], bf16, tag=f"g{r}")
                if no_gather:
                    nc.vector.memset(G[:], 0.01)
                else:
                    base = r * IDXC_R + c * NG_CHUNK * 8
                    nc.gpsimd.dma_gather(
                        G[:, :gic, :],
                        range_slice(tbl_halves, r),
                        idx_sb[:, base : base + gic * 8],
                        num_idxs=gic * 128,
                        num_idxs_reg=gic * 128,
                        elem_size=H,
                        queue_num=r,
                    )
                gt[r][c] = G

            if no_dve:
                s_fixed = cpool.tile([128, GPB, 128], bf16, name="s_fixed")
                nc.vector.memset(s_fixed[:], 0.007)

            for _rep in range(repeat):
                tbls = tbls_r[_rep]     # [layer][half]
                shs = shs_r[_rep]       # [layer][half]
                shv = [[nm_view(h) for h in pair] for pair in shs]
                gts = {l: [[None] * NCHUNKS_R for _ in range(NRANGE)]
                       for l in range(1, 5)}

                def stage_write(l, b, st, shv=shv):
                    if b < HBLK:
                        nc.sync.dma_start(shv[l][0][:, b, :], st[:])
                    else:
                        nc.sync.dma_start(shv[l][1][:, b - HBLK, :], st[:])

                def tbl_for(l, tbls=tbls):
                    # layer 1 gathers from the host-shipped x @ Wl1 table
                    if l == 1:
                        return [t1_d[0].ap(), t1_d[1].ap()]
                    return tbls[l - 1]

                def emit_head(l, gts=gts):
                    # first r0/r1 chunks of the next layer: their Pool
                    # dispatch + descgen overlaps the half-B AllGather.
                    for c in range(min(2, NCHUNKS_R)):
                        for r in (0, 1):
                            emit_gather(tbl_for(l), gts[l], r, c)

                # layer-1 gathers have no producer — start them immediately
                emit_head(1)

                # ---- layers
                for l in range(1, 5):
                    hT_prev = [xt_sb, hA, hB, hA][l - 1]
                    hT_next = [hA, hB, hA, None][l - 1]
                    gt = gts[l]
                    pending = []

                    def emit_block(b, l=l, gt=gt, hT_prev=hT_prev,
                                   hT_next=hT_next, pending=pending):
                        cols = slice(b * 128, (b + 1) * 128)
                        gsl = slice(b * GPB, (b + 1) * GPB)
                        if no_dve:
                            S = s_fixed
                        else:
                            S = spool.tile([128, GPB, 128], bf16, tag="s")
                            nc.vector.tensor_tensor(
                                S[:], iota20[:],
                                dr_sb[:, gsl].to_broadcast([128, GPB, 128]), EQ,
                            )
                            nc.vector.tensor_mul(
                                S[:], S[:],
                                iv_sb[:, gsl].to_broadcast([128, GPB, 128]),
                            )
                        ps = popool.tile([128, 128], fp32, tag="ps")
                        first = True
                        for r in range(NRANGE):
                            for k in range(capg):
                                g = b * capg + k
                                c, pos = divmod(g, NG_CHUNK)
                                G = gt[r][c]
                                if l < 4:
                                    nc.tensor.matmul(
                                        ps[:], G[:, pos, :],
                                        S[:, r * capg + k, :],
                                        start=first, stop=False,
                                    )
                                else:
                                    nc.tensor.matmul(
                                        ps[:], S[:, r * capg + k, :],
                                        G[:, pos, :],
                                        start=first, stop=False,
                                    )
                                first = False
                        if l < 4:
                            nc.tensor.matmul(
                                ps[:], w_sb[f"wr{l}"][:], hT_prev[:, cols],
                                start=False, stop=True,
                            )
                            nc.scalar.activation(
                                hT_next[:, cols], ps[:], AF.Relu,
                                bias=w_sb[f"bc{l}"][:],
                            )

                            def mk(b=b, l=l, hT_next=hT_next):
                                cols = slice(b * 128, (b + 1) * 128)
                                ps2 = ptpool.tile([128, 128], fp32, tag="ps2")
                                nc.tensor.matmul(
                                    ps2[:], hT_next[:, cols],
                                    w_sb[f"wl{l + 1}"][:],
                                    start=True, stop=True,
                                )
                                st = wpool.tile([128, 128], bf16, tag="st")
                                nc.scalar.copy(st[:], ps2[:])
                                stage_write(l, b, st)

                            pending.append(mk)
                            if len(pending) >= 2:
                                pending.pop(0)()
                        else:
                            nc.tensor.matmul(
                                ps[:], hT_prev[:, cols], w_sb["wr4"][:],
                                start=False, stop=True,
                            )
                            st = wpool.tile([128, 128], fp32, tag="ost")
                            nc.vector.tensor_add(st[:], ps[:], b4b_sb[:])
                            nc.sync.dma_start(out_v[:, b, :], st[:])

                    # r0/r1 gathers run 2 chunks ahead of r2/r3 (chunks 0-1
                    # were emitted at the previous layer boundary). Blocks
                    # are drained BEFORE the gathers that would rotate the
                    # pool buffers their matmuls read (r0/r1 need bufs=4).
                    nextb = 0
                    for c in range(NCHUNKS_R):
                        while (nextb < NBLK
                               and ((nextb + 1) * capg - 1) // NG_CHUNK
                               <= c - 1):
                            emit_block(nextb)
                            nextb += 1
                        if c + 2 < NCHUNKS_R:
                            emit_gather(tbl_for(l), gt, 0, c + 2)
                            emit_gather(tbl_for(l), gt, 1, c + 2)
                        emit_gather(tbl_for(l), gt, 2, c)
                        emit_gather(tbl_for(l), gt, 3, c)
                    while nextb < NBLK:
                        emit_block(nextb)
                        nextb += 1
                    while pending:
                        pending.pop(0)()
                    if l < 4:
                        allgather(shs[l][0], tbls[l][0])
                        emit_head(l + 1)
                        allgather(shs[l][1], tbls[l][1])

    nc.compile()
    return nc


def _get_program(capg, repeat=1, **kw):
    key = (capg, repeat, tuple(sorted(kw.items())))
    if key not in _compiled:
        _compiled[key] = _build_program(capg, repeat=repeat, **kw)
    return _compiled[key]


# ---------------------------------------------------------------- host side
def make_in_maps(x, edge_index, weights):
    from ml_dtypes import bfloat16

    src = np.asarray(edge_index[0]).astype(np.int64)
    dst = np.asarray(edge_index[1]).astype(np.int64)
    x = np.asarray(x, dtype=np.float32)

    cnt = np.bincount(dst, minlength=N).astype(np.float32)
    inv_full = (1.0 / np.maximum(cnt, 1.0)).astype(np.float32)

    core = dst // SHARD
    # half-major table layout: blocks 0-48 of every core first, then 49-97
    sloc = src % SHARD
    shalf = sloc // HALF_ROWS
    srow = shalf * HALF_TBL + (src // SHARD) * HALF_ROWS \
        + (sloc - shalf * HALF_ROWS)
    rng = srow // RANGE_ROWS
    loc = (srow % RANGE_ROWS).astype(np.int64)

    # pass 1: global CAPG (same static structure on every core)
    capg = 0
    percore = []
    for c in range(NCORES):
        m = core == c
        dloc = dst[m] - c * SHARD
        cell = (dloc >> 7) * NRANGE + rng[m]
        cnts = np.bincount(cell, minlength=NBLK * NRANGE)
        capg = max(capg, int(-(-cnts.max() // 128)))
        percore.append((m, dloc, cell, cnts))
    assert capg <= 10, f"unexpectedly unbalanced graph: capg={capg}"

    GPB = NRANGE * capg
    NGRP = NBLK * GPB
    SGRP_R = NBLK * capg

    iota = np.tile(np.arange(128, dtype=np.float32), (128, 1)).astype(bfloat16)

    # layer-1 table (x @ Wl1) in the half-major table row layout, replicated
    t1 = x @ np.asarray(weights["Wl1"], np.float32)
    nid = np.arange(N, dtype=np.int64)
    nloc = nid % SHARD
    nhalf = nloc // HALF_ROWS
    nrow = nhalf * HALF_TBL + (nid // SHARD) * HALF_ROWS \
        + (nloc - nhalf * HALF_ROWS)
    tbl1 = np.zeros((TBL_ROWS, H), np.float32)
    tbl1[nrow] = t1
    t1a = np.ascontiguousarray(tbl1[:HALF_TBL]).astype(bfloat16)
    t1b = np.ascontiguousarray(tbl1[HALF_TBL:]).astype(bfloat16)

    in_maps = []
    for c in range(NCORES):
        m, dloc, cell, cnts = percore[c]
        loc_c = loc[m]
        iv_e = inv_full[dst[m]]
        rel = (dloc & 127).astype(np.float32)

        order = np.lexsort((loc_c, cell))
        cell_o = cell[order]
        loc_o = loc_c[order]
        rel_o = rel[order]
        iv_o = iv_e[order]

        starts = np.concatenate([[0], np.cumsum(cnts)[:-1]])
        within = np.arange(cell_o.size, dtype=np.int64) - starts[cell_o]
        bb = cell_o // NRANGE
        rr = cell_o % NRANGE

        gi = np.zeros((NRANGE, SGRP_R * 128), np.int16)
        gi[rr, bb * (capg * 128) + within] = loc_o.astype(np.int16)

        k = within >> 7
        p = within & 127
        col = bb * GPB + rr * capg + k
        dr = np.full((128, NGRP), -1.0, np.float32)
        ivr = np.zeros((128, NGRP), np.float32)
        dr[p, col] = rel_o
        ivr[p, col] = iv_o

        idxw = np.ascontiguousarray(np.tile(np.concatenate(
            [gi[r].reshape(-1, 16).T for r in range(NRANGE)], axis=1
        ), (8, 1)))

        xt = np.zeros((F_IN, SHARD_P), np.float32)
        xt[:, :SHARD] = x[c * SHARD : (c + 1) * SHARD].T

        im = {
            "xt": xt.astype(bfloat16),
            "idx": idxw,
            "dr": dr.astype(bfloat16),
            "iv": ivr.astype(bfloat16),
            "iota": iota,
            "t1a": t1a,
            "t1b": t1b,
        }
        for l in range(1, 5):
            im[f"wl{l}"] = np.asarray(weights[f"Wl{l}"], np.float32).astype(bfloat16)
            im[f"wr{l}"] = np.asarray(weights[f"Wr{l}"], np.float32).astype(bfloat16)
        for l in range(1, 4):
            im[f"bc{l}"] = np.asarray(
                weights[f"b{l}"], np.float32).reshape(128, 1)
        im["b4b"] = np.ascontiguousarray(np.tile(
            np.asarray(weights["b4"], np.float32).reshape(1, 128), (128, 1)))
        in_maps.append(im)
    return in_maps, capg


def bench_exec(nc, in_maps, iters=5):
    """Mirror of bass2jax.run_bass_via_pjrt's multi-core path, but jits once,
    keeps inputs on device, and times repeated executions."""
    import time

    import jax
    import numpy as np_
    from jax.sharding import Mesh, PartitionSpec
    from jax.experimental.shard_map import shard_map

    from concourse import bass2jax, mybir

    bass2jax.install_neuronx_cc_hook()
    partition_name = (
        nc.partition_id_tensor.name if nc.partition_id_tensor else None
    )
    in_names, out_names, out_avals = [], [], []
    for alloc in nc.m.functions[0].allocations:
        if not isinstance(alloc, mybir.MemoryLocationSet):
            continue
        name = alloc.memorylocations[0].name
        if alloc.kind == "ExternalInput":
            if name != partition_name:
                in_names.append(name)
        elif alloc.kind == "ExternalOutput":
            out_names.append(name)
            out_avals.append(
                jax.core.ShapedArray(
                    tuple(alloc.tensor_shape), mybir.dt.np(alloc.dtype)
                )
            )
    n_params = len(in_names)
    all_in_names = list(in_names)
    if partition_name is not None:
        all_in_names.append(partition_name)

    def _body(*args):
        operands = list(args)
        if partition_name is not None:
            operands.append(bass2jax.partition_id_tensor())
        return tuple(
            bass2jax._bass_exec_p.bind(
                *operands,
                out_avals=tuple(out_avals),
                in_names=tuple(all_in_names),
                out_names=tuple(out_names),
                lowering_input_output_aliases=(),
                sim_require_finite=True,
                sim_require_nnan=True,
                nc=nc,
            )
        )

    n_cores = len(in_maps)
    devices = jax.devices()[:n_cores]
    mesh = Mesh(np_.asarray(devices), ("core",))
    fn = jax.jit(
        shard_map(
            _body,
            mesh=mesh,
            in_specs=(PartitionSpec("core"),) * n_params,
            out_specs=(PartitionSpec("core"),) * len(out_names),
            check_rep=False,
        ),
        keep_unused=True,
    )
    concat_in = [
        np_.concatenate([np_.asarray(in_maps[c][nm]) for c in range(n_cores)], axis=0)
        for nm in in_names
    ]
    dev_in = [jax.device_put(a) for a in concat_in]
    outs = fn(*dev_in)
    jax.block_until_ready(outs)
    times = []
    for _ in range(iters):
        t0 = time.perf_counter()
        outs = fn(*dev_in)
        jax.block_until_ready(outs)
        times.append(time.perf_counter() - t0)
    results = [
        {nm: np_.asarray(outs[i]).reshape(n_cores, *out_avals[i].shape)[c]
         for i, nm in enumerate(out_names)}
        for c in range(n_cores)
    ]
    return results, times


def kernel(x, edge_index, Wl1, Wr1, b1, Wl2, Wr2, b2, Wl3, Wr3, b3,
           Wl4, Wr4, b4, _trace=False, _trace_kwargs=None):
    from concourse.bass_utils import run_bass_kernel_spmd

    weights = {
        "Wl1": Wl1, "Wr1": Wr1, "b1": b1,
        "Wl2": Wl2, "Wr2": Wr2, "b2": b2,
        "Wl3": Wl3, "Wr3": Wr3, "b3": b3,
        "Wl4": Wl4, "Wr4": Wr4, "b4": b4,
    }
    in_maps, capg = make_in_maps(x, edge_index, weights)
    nc = _get_program(capg)
    res = run_bass_kernel_spmd(
        nc,
        in_maps,
        core_ids=list(range(NCORES)),
        trace=_trace,
        **(_trace_kwargs or {}),
    )
    shards = [res.results[c]["out"][:SHARD] for c in range(NCORES)]
    out = np.concatenate(shards, axis=0).astype(np.float32)
    if _trace:
        return out, res
    return out


# revision 51
# speedup vs baseline: 1.3261x; 1.3261x over previous
"""4-layer GraphSAGE (mean aggr) on 8 TRN2 NeuronCores — gather + PE segsum.

Strategy (dst-owner node partitioning, no scatter):
  - Nodes partitioned across 8 cores (12500 each, padded to 12544 = 98*128).
    Each core owns 98 dst blocks of 128 nodes.
  - Per layer, a replicated DRAM table holds TRANSFORMED features
    (table_l = h_{l-1} @ Wl_l, bf16), built shard-wise and AllGathered.
    Gathering transformed 128-wide rows makes the segment-sum directly
    produce mean@Wl.
  - Edges are grouped host-side by (dst block, src range) cells — 4 ranges
    of 25088 table rows keep gather indices int16. Each cell is padded to
    CAPG groups of 128 edges.
  - Aggregation is a tensor-engine segment-sum: for each 128-edge group, a
    one-hot matrix S[e, d] = (dst_rel[e] == d) * invdeg[e] is built on the
    vector engine (one is_equal against a replicated iota + one multiply),
    then matmul-accumulated into PSUM. No dma_scatter_add, no WAW chains.
  - Layers 1-3 accumulate feature-major psum [f, dst]: segsum (G as lhsT)
    + self term (Wr as lhsT, hT_prev as rhs), evicted with fused
    bias+ReLU (Act engine, per-partition bias) straight into an SBUF-resident
    feature-major hT. The next table (h @ Wl_{l+1}) is one more matmul per
    block (hT as lhsT) producing node-major rows — zero PE transposes.
  - Layer 4 accumulates node-major [dst, f] (S as lhsT); bias is a rank-1
    matmul (ones x b4^T); evicted fp32 to the output.
"""

import numpy as np

# ---------------------------------------------------------------- constants
NCORES = 8
N = 100000
E = 1600000
F_IN = 16
H = 128
SHARD = 12500            # real nodes owned per core
BLK = 128
NBLK = 98                # 98*128 = 12544
SHARD_P = NBLK * BLK     # padded shard rows
TBL_ROWS = NCORES * SHARD_P   # 100352
NRANGE = 4
RANGE_ROWS = TBL_ROWS // NRANGE  # 25088 (< 2**15)
# Table rows are stored half-major so each AllGather covers a contiguous
# half: srow = half*50176 + core*6272 + (local - half*6272), half = blocks
# 0-48 vs 49-97 of the owning core's shard. Ranges 0/1 = half A, 2/3 = half B.
HBLK = 49                     # dst blocks per table half
HALF_ROWS = HBLK * BLK        # 6272 rows contributed per core per half
HALF_TBL = NCORES * HALF_ROWS  # 50176 = 2 * RANGE_ROWS
# Gather groups (of 128 edges) per gather instruction. num_idxs > 1024 hangs
# the device (hard HW limit), so 8 groups = 1024 idxs per instruction (last
# chunk per range is partial). The Q7 descriptor generation (~7ns/desc,
# serial per queue) dominates gather cost, so the 4 ranges are spread over
# 4 SWDGE queues (separate Q7 core pairs).
NG_CHUNK = 8

_compiled = {}


# ---------------------------------------------------------------- program
def _build_program(capg, repeat=1, no_dve=False, no_cc=False, no_gather=False):
    import concourse.bacc as bacc
    import concourse.mybir as mybir
    import concourse.tile as tile

    fp32 = mybir.dt.float32
    bf16 = mybir.dt.bfloat16
    i16 = mybir.dt.int16
    AF = mybir.ActivationFunctionType
    EQ = mybir.AluOpType.is_equal

    GPB = NRANGE * capg          # groups per block (across ranges)
    NGRP = NBLK * GPB            # total groups (= dr/iv columns)
    SGRP_R = NBLK * capg         # groups per range stream
    NCHUNKS_R = -(-SGRP_R // NG_CHUNK)   # last chunk may be partial
    IDXC_R = SGRP_R * 8          # idx cols (16-wide wrap) per range

    def chunk_groups(c):
        return min(NG_CHUNK, SGRP_R - c * NG_CHUNK)

    nc = bacc.Bacc(
        "TRN2",
        target_bir_lowering=False,
        debug=False,
        enable_asserts=False,
        num_devices=NCORES,
        num_swdge_queues=4,
    )

    # -------- I/O declarations
    xt_d = nc.dram_tensor("xt", [F_IN, SHARD_P], bf16, kind="ExternalInput")
    idx_d = nc.dram_tensor("idx", [128, NRANGE * IDXC_R], i16, kind="ExternalInput")
    dr_d = nc.dram_tensor("dr", [128, NGRP], bf16, kind="ExternalInput")
    iv_d = nc.dram_tensor("iv", [128, NGRP], bf16, kind="ExternalInput")
    iota_d = nc.dram_tensor("iota", [128, 128], bf16, kind="ExternalInput")
    # layer-1 table (x @ Wl1, half-major row layout) is a pure function of
    # the inputs — precomputed host-side and shipped replicated, so layer-1
    # gathers have no on-device producer and start immediately.
    t1_d = [
        nc.dram_tensor("t1a", [HALF_TBL, H], bf16, kind="ExternalInput"),
        nc.dram_tensor("t1b", [HALF_TBL, H], bf16, kind="ExternalInput"),
    ]
    w_d = {}
    for l in range(1, 5):
        din = F_IN if l == 1 else H
        w_d[f"wl{l}"] = nc.dram_tensor(f"wl{l}", [din, H], bf16, kind="ExternalInput")
        w_d[f"wr{l}"] = nc.dram_tensor(f"wr{l}", [din, H], bf16, kind="ExternalInput")
    for l in range(1, 4):
        w_d[f"bc{l}"] = nc.dram_tensor(f"bc{l}", [128, 1], fp32, kind="ExternalInput")
    w_d["b4b"] = nc.dram_tensor("b4b", [128, 128], fp32, kind="ExternalInput")

    out_d = nc.dram_tensor("out", [SHARD_P, H], fp32, kind="ExternalOutput")

    with tile.TileContext(nc) as tc:
        with (
            tc.tile_pool(name="dram", bufs=1, space="DRAM") as dpool,
            tc.tile_pool(name="const", bufs=1) as cpool,
            tc.tile_pool(name="g0", bufs=4) as gp0,
            tc.tile_pool(name="g1", bufs=4) as gp1,
            tc.tile_pool(name="g2", bufs=3) as gp2,
            tc.tile_pool(name="g3", bufs=3) as gp3,
            tc.tile_pool(name="sp", bufs=3) as spool,
            tc.tile_pool(name="work", bufs=3) as wpool,
            tc.tile_pool(name="psum_a", bufs=4, space="PSUM") as popool,
            tc.tile_pool(name="psum_t", bufs=2, space="PSUM") as ptpool,
        ):
            gpools = [gp0, gp1, gp2, gp3]
            # Shared tiles may only be written by one instruction each, so
            # tables (AllGather outputs) are allocated per repeat. Each
            # layer's table is two Shared halves (separate AllGathers).
            tbls_r = [
                [[dpool.tile([HALF_TBL, H], bf16, addr_space="Shared",
                             name=f"tbl{l}{hn}_r{rep}")
                  for hn in ("a", "b")]
                 for l in range(4)]
                for rep in range(repeat)
            ]
            shs_r = [
                [[dpool.tile([HALF_ROWS, H], bf16, name=f"sh{l}{hn}_r{rep}")
                  for hn in ("a", "b")]
                 for l in range(4)]
                for rep in range(repeat)
            ]

            # -------- constants to SBUF
            idx_sb = cpool.tile([128, NRANGE * IDXC_R], i16, name="idx_sb")
            nc.sync.dma_start(idx_sb[:], idx_d.ap())
            dr_sb = cpool.tile([128, NGRP], bf16, name="dr_sb")
            nc.sync.dma_start(dr_sb[:], dr_d.ap())
            iv_sb = cpool.tile([128, NGRP], bf16, name="iv_sb")
            nc.sync.dma_start(iv_sb[:], iv_d.ap())
            iota1 = cpool.tile([128, 128], bf16, name="iota1")
            nc.sync.dma_start(iota1[:], iota_d.ap())
            iota20 = cpool.tile([128, GPB, 128], bf16, name="iota20")
            for j in range(GPB):
                nc.vector.tensor_copy(iota20[:, j, :], iota1[:])
            xt_sb = cpool.tile([F_IN, SHARD_P], bf16, name="xt_sb")
            nc.sync.dma_start(xt_sb[:], xt_d.ap())
            w_sb = {}
            for l in range(1, 5):
                din = F_IN if l == 1 else H
                for nm in (f"wl{l}", f"wr{l}"):
                    t = cpool.tile([din, H], bf16, name=f"{nm}_sb")
                    nc.sync.dma_start(t[:], w_d[nm].ap())
                    w_sb[nm] = t
            for l in range(1, 4):
                t = cpool.tile([128, 1], fp32, name=f"bc{l}_sb")
                nc.sync.dma_start(t[:], w_d[f"bc{l}"].ap())
                w_sb[f"bc{l}"] = t
            b4b_sb = cpool.tile([128, 128], fp32, name="b4b_sb")
            nc.sync.dma_start(b4b_sb[:], w_d["b4b"].ap())

            # feature-major hidden state, SBUF resident, ping-pong
            hA = cpool.tile([128, SHARD_P], bf16, name="hA")
            hB = cpool.tile([128, SHARD_P], bf16, name="hB")

            def nm_view(t):
                return t.rearrange("(b p) f -> p b f", p=128)

            out_v = nm_view(out_d.ap())

            def allgather(src, dst):
                if no_cc:
                    nc.sync.dma_start(dst[:HALF_ROWS, :], src[:, :])
                    return
                nc.gpsimd.collective_compute(
                    "AllGather",
                    mybir.AluOpType.bypass,
                    replica_groups=[list(range(NCORES))],
                    ins=[src.opt()],
                    outs=[dst.opt()],
                )

            def range_slice(tbl_halves, r):
                # ranges 0/1 live in half A, 2/3 in half B
                half = tbl_halves[r // 2]
                lo = (r % 2) * RANGE_ROWS
                return half[lo : lo + RANGE_ROWS, :]

            def emit_gather(tbl_halves, gt, r, c):
                gic = chunk_groups(c)
                G = gpools[r].tile([128, NG_CHUNK, 128], bf16, tag=f"g{r}")
                if no_gather:
                    nc.vector.memset(G[:], 0.01)
                else:
                    base = r * IDXC_R + c * NG_CHUNK * 8
                    nc.gpsimd.dma_gather(
                        G[:, :gic, :],
                        range_slice(tbl_halves, r),
                        idx_sb[:, base : base + gic * 8],
                        num_idxs=gic * 128,
                        num_idxs_reg=gic * 128,
                        elem_size=H,
                        queue_num=r,
                    )
                gt[r][c] = G

            if no_dve:
                s_fixed = cpool.tile([128, GPB, 128], bf16, name="s_fixed")
                nc.vector.memset(s_fixed[:], 0.007)

            for _rep in range(repeat):
                tbls = tbls_r[_rep]     # [layer][half]
                shs = shs_r[_rep]       # [layer][half]
                shv = [[nm_view(h) for h in pair] for pair in shs]
                gts = {l: [[None] * NCHUNKS_R for _ in range(NRANGE)]
                       for l in range(1, 5)}

                def stage_write(l, b, st, shv=shv):
                    if b < HBLK:
                        nc.sync.dma_start(shv[l][0][:, b, :], st[:])
                    else:
                        nc.sync.dma_start(shv[l][1][:, b - HBLK, :], st[:])

                def tbl_for(l, tbls=tbls):
                    # layer 1 gathers from the host-shipped x @ Wl1 table
                    if l == 1:
                        return [t1_d[0].ap(), t1_d[1].ap()]
                    return tbls[l - 1]

                def emit_head(l, gts=gts):
                    # first r0/r1 chunks of the next layer: their Pool
                    # dispatch + descgen overlaps the half-B AllGather.
                    for c in range(min(2, NCHUNKS_R)):
                        for r in (0, 1):
                            emit_gather(tbl_for(l), gts[l], r, c)

                # layer-1 gathers have no producer — start them immediately
                emit_head(1)

                # ---- layers
                for l in range(1, 5):
                    hT_prev = [xt_sb, hA, hB, hA][l - 1]
                    hT_next = [hA, hB, hA, None][l - 1]
                    gt = gts[l]
                    pending = []

                    def emit_block(b, l=l, gt=gt, hT_prev=hT_prev,
                                   hT_next=hT_next, pending=pending):
                        cols = slice(b * 128, (b + 1) * 128)
                        gsl = slice(b * GPB, (b + 1) * GPB)
                        if no_dve:
                            S = s_fixed
                        else:
                            S = spool.tile([128, GPB, 128], bf16, tag="s")
                            nc.vector.tensor_tensor(
                                S[:], iota20[:],
                                dr_sb[:, gsl].to_broadcast([128, GPB, 128]), EQ,
                            )
                            nc.vector.tensor_mul(
                                S[:], S[:],
                                iv_sb[:, gsl].to_broadcast([128, GPB, 128]),
                            )
                        ps = popool.tile([128, 128], fp32, tag="ps")
                        first = True
                        for r in range(NRANGE):
                            for k in range(capg):
                                g = b * capg + k
                                c, pos = divmod(g, NG_CHUNK)
                                G = gt[r][c]
                                if l < 4:
                                    nc.tensor.matmul(
                                        ps[:], G[:, pos, :],
                                        S[:, r * capg + k, :],
                                        start=first, stop=False,
                                    )
                                else:
                                    nc.tensor.matmul(
                                        ps[:], S[:, r * capg + k, :],
                                        G[:, pos, :],
                                        start=first, stop=False,
                                    )
                                first = False
                        if l < 4:
                            nc.tensor.matmul(
                                ps[:], w_sb[f"wr{l}"][:], hT_prev[:, cols],
                                start=False, stop=True,
                            )
                            nc.scalar.activation(
                                hT_next[:, cols], ps[:], AF.Relu,
                                bias=w_sb[f"bc{l}"][:],
                            )

                            def mk(b=b, l=l, hT_next=hT_next):
                                cols = slice(b * 128, (b + 1) * 128)
                                ps2 = ptpool.tile([128, 128], fp32, tag="ps2")
                                nc.tensor.matmul(
                                    ps2[:], hT_next[:, cols],
                                    w_sb[f"wl{l + 1}"][:],
                                    start=True, stop=True,
                                )
                                st = wpool.tile([128, 128], bf16, tag="st")
                                nc.scalar.copy(st[:], ps2[:])
                                stage_write(l, b, st)

                            pending.append(mk)
                            if len(pending) >= 2:
                                pending.pop(0)()
                        else:
                            nc.tensor.matmul(
                                ps[:], hT_prev[:, cols], w_sb["wr4"][:],
                                start=False, stop=True,
                            )
                            st = wpool.tile([128, 128], fp32, tag="ost")
                            nc.vector.tensor_add(st[:], ps[:], b4b_sb[:])
                            nc.sync.dma_start(out_v[:, b, :], st[:])

                    # r0/r1 gathers run 2 chunks ahead of r2/r3 (chunks 0-1
                    # were emitted at the previous layer boundary). Blocks
                    # are drained BEFORE the gathers that would rotate the
                    # pool buffers their matmuls read (r0/r1 need bufs=4).
                    nextb = 0
                    for c in range(NCHUNKS_R):
                        while (nextb < NBLK
                               and ((nextb + 1) * capg - 1) // NG_CHUNK
                               <= c - 1):
                            emit_block(nextb)
                            nextb += 1
                        if c + 2 < NCHUNKS_R:
                            emit_gather(tbl_for(l), gt, 0, c + 2)
                            emit_gather(tbl_for(l), gt, 1, c + 2)
                        emit_gather(tbl_for(l), gt, 2, c)
                        emit_gather(tbl_for(l), gt, 3, c)
                    while nextb < NBLK:
                        emit_block(nextb)
                        nextb += 1
                    while pending:
                        pending.pop(0)()
                    if l < 4:
                        allgather(shs[l][0], tbls[l][0])
                        emit_head(l + 1)
                        allgather(shs[l][1], tbls[l][1])

    nc.compile()
    return nc


def _get_program(capg, repeat=1, **kw):
    key = (capg, repeat, tuple(sorted(kw.items())))
    if key not in _compiled:
        _compiled[key] = _build_program(capg, repeat=repeat, **kw)
    return _compiled[key]


# ---------------------------------------------------------------- host side
def make_in_maps(x, edge_index, weights):
    from ml_dtypes import bfloat16

    src = np.asarray(edge_index[0]).astype(np.int64)
    dst = np.asarray(edge_index[1]).astype(np.int64)
    x = np.asarray(x, dtype=np.float32)

    cnt = np.bincount(dst, minlength=N).astype(np.float32)
    inv_full = (1.0 / np.maximum(cnt, 1.0)).astype(np.float32)

    core = dst // SHARD
    # half-major table layout: blocks 0-48 of every core first, then 49-97
    sloc = src % SHARD
    shalf = sloc // HALF_ROWS
    srow = shalf * HALF_TBL + (src // SHARD) * HALF_ROWS \
        + (sloc - shalf * HALF_ROWS)
    rng = srow // RANGE_ROWS
    loc = (srow % RANGE_ROWS).astype(np.int64)

    # pass 1: global CAPG (same static structure on every core)
    capg = 0
    percore = []
    for c in range(NCORES):
        m = core == c
        dloc = dst[m] - c * SHARD
        cell = (dloc >> 7) * NRANGE + rng[m]
        cnts = np.bincount(cell, minlength=NBLK * NRANGE)
        capg = max(capg, int(-(-cnts.max() // 128)))
        percore.append((m, dloc, cell, cnts))
    assert capg <= 10, f"unexpectedly unbalanced graph: capg={capg}"

    GPB = NRANGE * capg
    NGRP = NBLK * GPB
    SGRP_R = NBLK * capg

    iota = np.tile(np.arange(128, dtype=np.float32), (128, 1)).astype(bfloat16)

    # layer-1 table (x @ Wl1) in the half-major table row layout, replicated
    t1 = x @ np.asarray(weights["Wl1"], np.float32)
    nid = np.arange(N, dtype=np.int64)
    nloc = nid % SHARD
    nhalf = nloc // HALF_ROWS
    nrow = nhalf * HALF_TBL + (nid // SHARD) * HALF_ROWS \
        + (nloc - nhalf * HALF_ROWS)
    tbl1 = np.zeros((TBL_ROWS, H), np.float32)
    tbl1[nrow] = t1
    t1a = np.ascontiguousarray(tbl1[:HALF_TBL]).astype(bfloat16)
    t1b = np.ascontiguousarray(tbl1[HALF_TBL:]).astype(bfloat16)

    in_maps = []
    for c in range(NCORES):
        m, dloc, cell, cnts = percore[c]
        loc_c = loc[m]
        iv_e = inv_full[dst[m]]
        rel = (dloc & 127).astype(np.float32)

        order = np.lexsort((loc_c, cell))
        cell_o = cell[order]
        loc_o = loc_c[order]
        rel_o = rel[order]
        iv_o = iv_e[order]

        starts = np.concatenate([[0], np.cumsum(cnts)[:-1]])
        within = np.arange(cell_o.size, dtype=np.int64) - starts[cell_o]
        bb = cell_o // NRANGE
        rr = cell_o % NRANGE

        gi = np.zeros((NRANGE, SGRP_R * 128), np.int16)
        gi[rr, bb * (capg * 128) + within] = loc_o.astype(np.int16)

        k = within >> 7
        p = within & 127
        col = bb * GPB + rr * capg + k
        dr = np.full((128, NGRP), -1.0, np.float32)
        ivr = np.zeros((128, NGRP), np.float32)
        dr[p, col] = rel_o
        ivr[p, col] = iv_o

        idxw = np.ascontiguousarray(np.tile(np.concatenate(
            [gi[r].reshape(-1, 16).T for r in range(NRANGE)], axis=1
        ), (8, 1)))

        xt = np.zeros((F_IN, SHARD_P), np.float32)
        xt[:, :SHARD] = x[c * SHARD : (c + 1) * SHARD].T

        im = {
            "xt": xt.astype(bfloat16),
            "idx": idxw,
            "dr": dr.astype(bfloat16),
            "iv": ivr.astype(bfloat16),
            "iota": iota,
            "t1a": t1a,
            "t1b": t1b,
        }
        for l in range(1, 5):
            im[f"wl{l}"] = np.asarray(weights[f"Wl{l}"], np.float32).astype(bfloat16)
            im[f"wr{l}"] = np.asarray(weights[f"Wr{l}"], np.float32).astype(bfloat16)
        for l in range(1, 4):
            im[f"bc{l}"] = np.asarray(
                weights[f"b{l}"], np.float32).reshape(128, 1)
        im["b4b"] = np.ascontiguousarray(np.tile(
            np.asarray(weights["b4"], np.float32).reshape(1, 128), (128, 1)))
        in_maps.append(im)
    return in_maps, capg


def bench_exec(nc, in_maps, iters=5):
    """Mirror of bass2jax.run_bass_via_pjrt's multi-core path, but jits once,
    keeps inputs on device, and times repeated executions."""
    import time

    import jax
    import numpy as np_
    from jax.sharding import Mesh, PartitionSpec
    from jax.experimental.shard_map import shard_map

    from concourse import bass2jax, mybir

    bass2jax.install_neuronx_cc_hook()
    partition_name = (
        nc.partition_id_tensor.name if nc.partition_id_tensor else None
    )
    in_names, out_names, out_avals = [], [], []
    for alloc in nc.m.functions[0].allocations:
        if not isinstance(alloc, mybir.MemoryLocationSet):
            continue
        name = alloc.memorylocations[0].name
        if alloc.kind == "ExternalInput":
            if name != partition_name:
                in_names.append(name)
        elif alloc.kind == "ExternalOutput":
            out_names.append(name)
            out_avals.append(
                jax.core.ShapedArray(
                    tuple(alloc.tensor_shape), mybir.dt.np(alloc.dtype)
                )
            )
    n_params = len(in_names)
    all_in_names = list(in_names)
    if partition_name is not None:
        all_in_names.append(partition_name)

    def _body(*args):
        operands = list(args)
        if partition_name is not None:
            operands.append(bass2jax.partition_id_tensor())
        return tuple(
            bass2jax._bass_exec_p.bind(
                *operands,
                out_avals=tuple(out_avals),
                in_names=tuple(all_in_names),
                out_names=tuple(out_names),
                lowering_input_output_aliases=(),
                sim_require_finite=True,
                sim_require_nnan=True,
                nc=nc,
            )
        )

    n_cores = len(in_maps)
    devices = jax.devices()[:n_cores]
    mesh = Mesh(np_.asarray(devices), ("core",))
    fn = jax.jit(
        shard_map(
            _body,
            mesh=mesh,
            in_specs=(PartitionSpec("core"),) * n_params,
            out_specs=(PartitionSpec("core"),) * len(out_names),
            check_rep=False,
        ),
        keep_unused=True,
    )
    concat_in = [
        np_.concatenate([np_.asarray(in_maps[c][nm]) for c in range(n_cores)], axis=0)
        for nm in in_names
    ]
    dev_in = [jax.device_put(a) for a in concat_in]
    outs = fn(*dev_in)
    jax.block_until_ready(outs)
    times = []
    for _ in range(iters):
        t0 = time.perf_counter()
        outs = fn(*dev_in)
        jax.block_until_ready(outs)
        times.append(time.perf_counter() - t0)
    results = [
        {nm: np_.asarray(outs[i]).reshape(n_cores, *out_avals[i].shape)[c]
         for i, nm in enumerate(out_names)}
        for c in range(n_cores)
    ]
    return results, times


def kernel(x, edge_index, Wl1, Wr1, b1, Wl2, Wr2, b2, Wl3, Wr3, b3,
           Wl4, Wr4, b4, _trace=False, _trace_kwargs=None):
    from concourse.bass_utils import run_bass_kernel_spmd

    weights = {
        "Wl1": Wl1, "Wr1": Wr1, "b1": b1,
        "Wl2": Wl2, "Wr2": Wr2, "b2": b2,
        "Wl3": Wl3, "Wr3": Wr3, "b3": b3,
        "Wl4": Wl4, "Wr4": Wr4, "b4": b4,
    }
    in_maps, capg = make_in_maps(x, edge_index, weights)
    nc = _get_program(capg)
    res = run_bass_kernel_spmd(
        nc,
        in_maps,
        core_ids=list(range(NCORES)),
        trace=_trace,
        **(_trace_kwargs or {}),
    )
    shards = [res.results[c]["out"][:SHARD] for c in range(NCORES)]
    out = np.concatenate(shards, axis=0).astype(np.float32)
    if _trace:
        return out, res
    return out
